# revision 13
# baseline (speedup 1.0000x reference)
"""Trainium2 Bass kernel: 3x3 stride-1 pad-1 conv2d, N=16,Cin=64,Cout=128,H=W=224.

Sharding: data-parallel over batch: 8 cores x 2 images each.

Per-core algorithm:
  - x lives in SBUF bands of R output rows per image, stored UNPADDED and
    contiguous: band row i = image row y0-1+i, flat [64, (R+2)*224] (+2
    guard elems, data at offset 1). Contiguous src+dst -> single ~30KB DMA
    descriptor per partition. partitions 0-63 = img0, 64-127 = img1.
  - conv = sum over 9 taps (dr,dc) of fp16 matmuls with flat-shifted rhs:
      psum[co, f] += w[ci, tap, co].T @ band[ci, 1 + c*512 + dr*224 + dc-1 ...]
    K=64 (Cin) partitions, M=128 (Cout), N=512 (one PSUM bank).
    fp16 in, fp22 multiply, fp32 accumulate; x is cast to fp16 on host
    (halves input HBM traffic), weights are fp16.
  - img0 matmuls use PE rows 0-63, img1 rows 64-127 (tile_position derived
    from base partitions) -> the two streams run concurrently in disjoint
    row-groups of the systolic array (measured 116 ns/MM at N=512).
  - Column wrap: the flat shift makes out columns x=0 (dc=0 taps) and
    x=223 (dc=2 taps) read the neighboring row's edge pixel. Those two
    output columns are recomputed correctly per band with 6 small
    edge matmuls (valid taps only, strided rhs) and overwrite the staged
    output before DMA-out.
  - PSUM chunk [128, 512] evicted to SBUF staging with fused bias add
    (DVE mostly, ACT for some chunks); staged band DMA'd out on the
    scalar-engine queue so input loads (sync queue) and output stores
    overlap.
"""

import numpy as np

N_IMG, C_IN, C_OUT, KS, H, W = 16, 64, 128, 3, 224, 224
N_CORES = 8
IMGS_PER_CORE = N_IMG // N_CORES  # 2
R = 32  # output rows per band
CHUNK = 512  # flat pixels per PSUM chunk
TAPS = [(dr, dc) for dr in range(KS) for dc in range(KS)]


def build_conv_program(h=H, w=W, r=R, evict_split=2, out_bf16=False):
    import concourse.bacc as bacc
    import concourse.mybir as mybir
    import concourse.tile as tile

    n_bands = h // r
    flat = r * w
    n_chunk = flat // CHUNK
    assert r * w % CHUNK == 0 and h % r == 0
    band_len = (r + 2) * w + 2  # +2 guard elems, data at offset 1
    f32 = mybir.dt.float32
    f32r = mybir.dt.float32r
    f16 = mybir.dt.float16
    odt = mybir.dt.bfloat16 if out_bf16 else f32

    nc = bacc.Bacc("TRN2", target_bir_lowering=False)

    x_d = nc.dram_tensor("x", [IMGS_PER_CORE, C_IN, h, w], f16, kind="ExternalInput")
    w_d = nc.dram_tensor("w", [C_IN, 9, C_OUT], f16, kind="ExternalInput")
    b_d = nc.dram_tensor("bias", [C_OUT, 1], f32, kind="ExternalInput")
    zz_d = nc.dram_tensor("zz", [128, w], f16, kind="ExternalInput")
    out_d = nc.dram_tensor(
        "out", [IMGS_PER_CORE, C_OUT, h, w], odt, kind="ExternalOutput"
    )

    with tile.TileContext(nc) as tc:
        with (
            tc.tile_pool(name="const", bufs=1) as const_pool,
            tc.tile_pool(name="xband", bufs=2) as x_pool,
            tc.tile_pool(name="outs", bufs=2) as o_pool,
            tc.tile_pool(name="psum", bufs=6, space="PSUM") as p_pool,
        ):
            # fp16 weights: half the LDWEIGHTS time of 4-byte fp32r, so the
            # per-matmul weight load hides under the N=512 stream. PE
            # upconverts to fp22 internally; accumulation stays fp32.
            w_sb = const_pool.tile([128, 9, C_OUT], f16)
            nc.sync.dma_start(out=w_sb[0:64], in_=w_d[:])
            nc.sync.dma_start(out=w_sb[64:128], in_=w_d[:])
            bias_sb = const_pool.tile([C_OUT, 1], f32)
            nc.sync.dma_start(out=bias_sb[:], in_=b_d[:])
            zzr = zz_d[:]

            bands = [
                x_pool.tile([128, band_len], f16, tag="band", name=f"band{i}")
                for i in range(2)
            ]

            for b in range(n_bands):
                y0 = b * r
                bt = bands[b % 2]
                rows_lo = max(y0 - 1, 0)
                rows_hi = min(y0 + r + 1, h)
                dst_r0 = rows_lo - (y0 - 1)
                nrows = rows_hi - rows_lo
                if b == 0:
                    # top halo row of the image is zero
                    nc.sync.dma_start(out=bt[:, 1 : 1 + w], in_=zzr[:, :])
                if b == n_bands - 1:
                    # bottom halo row is zero (buffer may hold stale data)
                    nc.sync.dma_start(
                        out=bt[:, 1 + (r + 1) * w : 1 + (r + 2) * w], in_=zzr[:, :]
                    )
                n_pieces = 4 if b == 0 else 1
                for img in range(IMGS_PER_CORE):
                    p0 = img * 64
                    for pc in range(n_pieces):
                        r_a = pc * nrows // n_pieces
                        r_b = (pc + 1) * nrows // n_pieces
                        nc.sync.dma_start(
                            out=bt[
                                p0 : p0 + 64,
                                1 + (dst_r0 + r_a) * w : 1 + (dst_r0 + r_b) * w,
                            ],
                            in_=x_d[img, :, rows_lo + r_a : rows_lo + r_b, :],
                        )

                ost = [
                    o_pool.tile(
                        [C_OUT, flat], odt, tag=f"ost{img}", name=f"ost{img}_{b}"
                    )
                    for img in range(IMGS_PER_CORE)
                ]

                edge_evicts = []

                def emit_edges(img, side):
                    p0 = img * 64
                    pse = p_pool.tile(
                        [C_OUT, r],
                        f32,
                        tag="pse",
                        bufs=2,
                        name=f"pse{img}_{side}_{b}",
                    )
                    dcs = (1, 2) if side == 0 else (0, 1)
                    n6 = 0
                    for dr in range(3):
                        for dc in dcs:
                            t = dr * 3 + dc
                            xcol_in = (dc - 1) if side == 0 else (w - 2 + dc)
                            base = 1 + dr * w + xcol_in
                            rhs = bt[p0 : p0 + 64, base : base + (r - 1) * w + 1 : w]
                            nc.tensor.matmul(
                                pse[:],
                                w_sb[p0 : p0 + 64, t, :],
                                rhs,
                                start=(n6 == 0),
                                stop=(n6 == 5),
                            )
                            n6 += 1
                    xcol = 0 if side == 0 else w - 1
                    dst = ost[img][:].rearrange("p (a b) -> p a b", b=w)[
                        :, :, xcol : xcol + 1
                    ]

                    def ev(dst=dst, pse=pse):
                        nc.vector.tensor_scalar_add(
                            dst, pse[:].unsqueeze(2), bias_sb[:]
                        )

                    edge_evicts.append(ev)

                edge_sched = {
                    n_chunk - 5: (0, 0),
                    n_chunk - 4: (0, 1),
                    n_chunk - 3: (1, 0),
                    n_chunk - 2: (1, 1),
                }

                for c in range(n_chunk):
                    if c in edge_sched:
                        emit_edges(*edge_sched[c])
                    ps = [
                        p_pool.tile(
                            [C_OUT, CHUNK],
                            f32,
                            tag="ps",
                            bufs=6,
                            name=f"ps{i}_{b}_{c}",
                        )
                        for i in range(2)
                    ]
                    for t, (dr, dc) in enumerate(TAPS):
                        st = t == 0
                        sp = t == 8
                        base = 1 + c * CHUNK + dr * w + dc - 1
                        for img in range(IMGS_PER_CORE):
                            p0 = img * 64
                            nc.tensor.matmul(
                                ps[img][:],
                                w_sb[p0 : p0 + 64, t, :],
                                bt[p0 : p0 + 64, base : base + CHUNK],
                                start=st,
                                stop=sp,
                            )
                    for img in range(IMGS_PER_CORE):
                        dst = ost[img][:, c * CHUNK : (c + 1) * CHUNK]
                        if (c % 4) < evict_split:
                            nc.vector.tensor_scalar_add(dst, ps[img][:], bias_sb[:])
                        else:
                            nc.scalar.add(dst, ps[img][:], bias_sb[:])

                # Edge-column evictions overwrite columns 0/w-1 of the
                # staged band, so they run after all chunk evictions.
                for ev in edge_evicts:
                    ev()

                # Last band: split the store so it overlaps the trailing
                # evictions instead of serializing after them.
                n_out = 2 if b == n_bands - 1 else 1
                for img in range(IMGS_PER_CORE):
                    for oc in range(n_out):
                        r_a = oc * r // n_out
                        r_b = (oc + 1) * r // n_out
                        nc.scalar.dma_start(
                            out=out_d[img, :, y0 + r_a : y0 + r_b, :],
                            in_=ost[img][:, r_a * w : r_b * w],
                        )

    nc.compile()
    return nc


def prep_weight(weight: np.ndarray) -> np.ndarray:
    # [C_OUT, C_IN, 3, 3] -> [C_IN, 9, C_OUT]
    return np.ascontiguousarray(weight.transpose(1, 2, 3, 0).reshape(C_IN, 9, C_OUT))


def run_conv(x, weight, bias, trace=False, h=H, r=R, out_bf16=False, evict_split=2):
    """x [16,64,224,224] f32. Returns (out [16,128,224,224] f32, results)."""
    from concourse.bass_utils import run_bass_kernel_spmd

    x = np.asarray(x, dtype=np.float32).astype(np.float16)
    w_t = prep_weight(np.asarray(weight, dtype=np.float32)).astype(np.float16)
    b_t = np.ascontiguousarray(np.asarray(bias, dtype=np.float32).reshape(C_OUT, 1))

    nc = build_conv_program(h=h, r=r, out_bf16=out_bf16, evict_split=evict_split)
    zz_np = np.zeros((128, W), np.float16)
    in_maps = [
        {
            "x": np.ascontiguousarray(x[i * IMGS_PER_CORE : (i + 1) * IMGS_PER_CORE]),
            "w": w_t,
            "bias": b_t,
            "zz": zz_np,
        }
        for i in range(N_CORES)
    ]
    res = run_bass_kernel_spmd(nc, in_maps, core_ids=list(range(N_CORES)), trace=trace)
    out = np.concatenate([r_["out"] for r_ in res.results], axis=0)
    if out.dtype != np.float32:
        out = out.astype(np.float32)
    return out, res


def kernel(**inputs) -> np.ndarray:
    out, _ = run_conv(inputs["x"], inputs["weight"], inputs["bias"])
    return out


# revision 14
# speedup vs baseline: 1.0653x; 1.0653x over previous
"""Trainium2 Bass kernel: 3x3 stride-1 pad-1 conv2d, N=16,Cin=64,Cout=128,H=W=224.

Sharding: data-parallel over batch: 8 cores x 2 images each.

Per-core algorithm:
  - x lives in SBUF bands of R output rows per image, stored UNPADDED and
    contiguous: band row i = image row y0-1+i, flat [64, (R+2)*224] (+2
    guard elems, data at offset 1). Contiguous src+dst -> single ~30KB DMA
    descriptor per partition. partitions 0-63 = img0, 64-127 = img1.
  - conv = sum over 9 taps (dr,dc) of fp16 matmuls with flat-shifted rhs:
      psum[co, f] += w[ci, tap, co].T @ band[ci, 1 + c*512 + dr*224 + dc-1 ...]
    K=64 (Cin) partitions, M=128 (Cout), N=512 (one PSUM bank).
    fp16 in, fp22 multiply, fp32 accumulate; x is cast to fp16 on host
    (halves input HBM traffic), weights are fp16.
  - img0 matmuls use PE rows 0-63, img1 rows 64-127 (tile_position derived
    from base partitions) -> the two streams run concurrently in disjoint
    row-groups of the systolic array (measured 116 ns/MM at N=512).
  - Column wrap: the flat shift makes out columns x=0 (dc=0 taps) and
    x=223 (dc=2 taps) read the neighboring row's edge pixel. Those two
    output columns are recomputed correctly per band with 6 small
    edge matmuls (valid taps only, strided rhs) and overwrite the staged
    output before DMA-out.
  - PSUM chunk [128, 512] evicted to SBUF staging with fused bias add
    (DVE mostly, ACT for some chunks); staged band DMA'd out on the
    scalar-engine queue so input loads (sync queue) and output stores
    overlap.
"""

import numpy as np

N_IMG, C_IN, C_OUT, KS, H, W = 16, 64, 128, 3, 224, 224
N_CORES = 8
IMGS_PER_CORE = N_IMG // N_CORES  # 2
R = 32  # output rows per band
CHUNK = 512  # flat pixels per PSUM chunk
TAPS = [(dr, dc) for dr in range(KS) for dc in range(KS)]


def build_conv_program(h=H, w=W, r=R, evict_split=3, out_bf16=False):
    import concourse.bacc as bacc
    import concourse.mybir as mybir
    import concourse.tile as tile

    n_bands = h // r
    flat = r * w
    n_chunk = flat // CHUNK
    assert r * w % CHUNK == 0 and h % r == 0
    band_len = (r + 2) * w + 2  # +2 guard elems, data at offset 1
    f32 = mybir.dt.float32
    f32r = mybir.dt.float32r
    f16 = mybir.dt.float16
    odt = mybir.dt.bfloat16 if out_bf16 else f32

    nc = bacc.Bacc("TRN2", target_bir_lowering=False)

    x_d = nc.dram_tensor("x", [IMGS_PER_CORE, C_IN, h, w], f16, kind="ExternalInput")
    w_d = nc.dram_tensor("w", [C_IN, 9, C_OUT], f16, kind="ExternalInput")
    b_d = nc.dram_tensor("bias", [C_OUT, 1], f32, kind="ExternalInput")
    zz_d = nc.dram_tensor("zz", [128, w], f16, kind="ExternalInput")
    out_d = nc.dram_tensor(
        "out", [IMGS_PER_CORE, C_OUT, h, w], odt, kind="ExternalOutput"
    )

    with tile.TileContext(nc) as tc:
        with (
            tc.tile_pool(name="const", bufs=1) as const_pool,
            tc.tile_pool(name="xband", bufs=2) as x_pool,
            tc.tile_pool(name="outs", bufs=2) as o_pool,
            tc.tile_pool(name="psum", bufs=6, space="PSUM") as p_pool,
        ):
            # fp16 weights: half the LDWEIGHTS time of 4-byte fp32r, so the
            # per-matmul weight load hides under the N=512 stream. PE
            # upconverts to fp22 internally; accumulation stays fp32.
            w_sb = const_pool.tile([128, 9, C_OUT], f16)
            nc.sync.dma_start(out=w_sb[0:64], in_=w_d[:])
            nc.sync.dma_start(out=w_sb[64:128], in_=w_d[:])
            bias_sb = const_pool.tile([C_OUT, 1], f32)
            nc.sync.dma_start(out=bias_sb[:], in_=b_d[:])
            zzr = zz_d[:]

            bands = [
                x_pool.tile([128, band_len], f16, tag="band", name=f"band{i}")
                for i in range(2)
            ]

            for b in range(n_bands):
                y0 = b * r
                bt = bands[b % 2]
                rows_lo = max(y0 - 1, 0)
                rows_hi = min(y0 + r + 1, h)
                dst_r0 = rows_lo - (y0 - 1)
                nrows = rows_hi - rows_lo
                if b == 0:
                    # top halo row of the image is zero
                    nc.sync.dma_start(out=bt[:, 1 : 1 + w], in_=zzr[:, :])
                if b == n_bands - 1:
                    # bottom halo row is zero (buffer may hold stale data)
                    nc.sync.dma_start(
                        out=bt[:, 1 + (r + 1) * w : 1 + (r + 2) * w], in_=zzr[:, :]
                    )
                n_pieces = 4 if b == 0 else 1
                for img in range(IMGS_PER_CORE):
                    p0 = img * 64
                    for pc in range(n_pieces):
                        r_a = pc * nrows // n_pieces
                        r_b = (pc + 1) * nrows // n_pieces
                        nc.sync.dma_start(
                            out=bt[
                                p0 : p0 + 64,
                                1 + (dst_r0 + r_a) * w : 1 + (dst_r0 + r_b) * w,
                            ],
                            in_=x_d[img, :, rows_lo + r_a : rows_lo + r_b, :],
                        )

                ost = [
                    o_pool.tile(
                        [C_OUT, flat], odt, tag=f"ost{img}", name=f"ost{img}_{b}"
                    )
                    for img in range(IMGS_PER_CORE)
                ]

                edge_evicts = []

                def emit_edges(img, side):
                    p0 = img * 64
                    pse = p_pool.tile(
                        [C_OUT, r],
                        f32,
                        tag="pse",
                        bufs=2,
                        name=f"pse{img}_{side}_{b}",
                    )
                    dcs = (1, 2) if side == 0 else (0, 1)
                    n6 = 0
                    for dr in range(3):
                        for dc in dcs:
                            t = dr * 3 + dc
                            xcol_in = (dc - 1) if side == 0 else (w - 2 + dc)
                            base = 1 + dr * w + xcol_in
                            rhs = bt[p0 : p0 + 64, base : base + (r - 1) * w + 1 : w]
                            nc.tensor.matmul(
                                pse[:],
                                w_sb[p0 : p0 + 64, t, :],
                                rhs,
                                start=(n6 == 0),
                                stop=(n6 == 5),
                            )
                            n6 += 1
                    xcol = 0 if side == 0 else w - 1
                    dst = ost[img][:].rearrange("p (a b) -> p a b", b=w)[
                        :, :, xcol : xcol + 1
                    ]

                    def ev(dst=dst, pse=pse):
                        nc.vector.tensor_scalar_add(
                            dst, pse[:].unsqueeze(2), bias_sb[:]
                        )

                    edge_evicts.append(ev)

                for c in range(n_chunk):
                    ps = [
                        p_pool.tile(
                            [C_OUT, CHUNK],
                            f32,
                            tag="ps",
                            bufs=6,
                            name=f"ps{i}_{b}_{c}",
                        )
                        for i in range(2)
                    ]
                    for t, (dr, dc) in enumerate(TAPS):
                        st = t == 0
                        sp = t == 8
                        base = 1 + c * CHUNK + dr * w + dc - 1
                        for img in range(IMGS_PER_CORE):
                            p0 = img * 64
                            nc.tensor.matmul(
                                ps[img][:],
                                w_sb[p0 : p0 + 64, t, :],
                                bt[p0 : p0 + 64, base : base + CHUNK],
                                start=st,
                                stop=sp,
                            )
                    for img in range(IMGS_PER_CORE):
                        dst = ost[img][:, c * CHUNK : (c + 1) * CHUNK]
                        if (c % 4) < evict_split:
                            nc.vector.tensor_scalar_add(dst, ps[img][:], bias_sb[:])
                        else:
                            nc.scalar.add(dst, ps[img][:], bias_sb[:])

                # Edge matmul groups at band end; their evictions overwrite
                # columns 0/w-1 of the staged band after all chunk evictions.
                for img in range(IMGS_PER_CORE):
                    for side in range(2):
                        emit_edges(img, side)
                for ev in edge_evicts:
                    ev()

                # Last band: split the store so it overlaps the trailing
                # evictions instead of serializing after them.
                n_out = 2 if b == n_bands - 1 else 1
                for img in range(IMGS_PER_CORE):
                    for oc in range(n_out):
                        r_a = oc * r // n_out
                        r_b = (oc + 1) * r // n_out
                        nc.scalar.dma_start(
                            out=out_d[img, :, y0 + r_a : y0 + r_b, :],
                            in_=ost[img][:, r_a * w : r_b * w],
                        )

    nc.compile()
    return nc


def prep_weight(weight: np.ndarray) -> np.ndarray:
    # [C_OUT, C_IN, 3, 3] -> [C_IN, 9, C_OUT]
    return np.ascontiguousarray(weight.transpose(1, 2, 3, 0).reshape(C_IN, 9, C_OUT))


def run_conv(x, weight, bias, trace=False, h=H, r=R, out_bf16=False, evict_split=3):
    """x [16,64,224,224] f32. Returns (out [16,128,224,224] f32, results)."""
    from concourse.bass_utils import run_bass_kernel_spmd

    x = np.asarray(x, dtype=np.float32).astype(np.float16)
    w_t = prep_weight(np.asarray(weight, dtype=np.float32)).astype(np.float16)
    b_t = np.ascontiguousarray(np.asarray(bias, dtype=np.float32).reshape(C_OUT, 1))

    nc = build_conv_program(h=h, r=r, out_bf16=out_bf16, evict_split=evict_split)
    zz_np = np.zeros((128, W), np.float16)
    in_maps = [
        {
            "x": np.ascontiguousarray(x[i * IMGS_PER_CORE : (i + 1) * IMGS_PER_CORE]),
            "w": w_t,
            "bias": b_t,
            "zz": zz_np,
        }
        for i in range(N_CORES)
    ]
    res = run_bass_kernel_spmd(nc, in_maps, core_ids=list(range(N_CORES)), trace=trace)
    out = np.concatenate([r_["out"] for r_ in res.results], axis=0)
    if out.dtype != np.float32:
        out = out.astype(np.float32)
    return out, res


def kernel(**inputs) -> np.ndarray:
    out, _ = run_conv(inputs["x"], inputs["weight"], inputs["bias"])
    return out
